# revision 25
# baseline (speedup 1.0000x reference)
"""GCN layer kernel for Trainium2 (Bass/Tile), data-parallel over batch.

Reference computation (per batch element):
    deg = A.sum(-1); d = deg ** -0.5
    t   = X @ W.T + b
    out = relu(diag(d) @ A @ diag(d) @ t)

Per-core mapping (8 cores, one batch element each). Host-side staging is
layout/dtype only (transposes + bf16 rounding, the same rounding the device
matmul path would apply); all model arithmetic (degree, normalization,
matmuls, bias, relu) runs on device:
  - A is staged twice in bf16: AT (transposed, the matmul stationary) and
    AN (natural, for the on-device degree row-sums). Streaming over the
    contraction index k, AT row-tile k provides the stationary chunks for
    ALL 16 output tiles, so each step runs a uniform batch of 16 products
    (k, mu) — no triangular schedule and no on-device transposes.
  - deg row-sums split DVE/ACT (accum_out) from AN tiles; d = sqrt(1/deg).
  - t = X @ W.T + b in bf16 from host-staged XT/WT, two chains per PSUM
    bank (8 wide drains split ACT/DVE); the bias is folded in as a K=1
    ones x b product initializing each group. Pairs 0-1 run in the head
    (doubling as PE warm-up for the HAM clock gate, topped up by a few
    identity matmuls); pairs 2-7 interleave into the first stream steps.
    y[k] = d[k] * t[k] rounded to bf16 by ACT.
  - All 16 output chains accumulate in PSUM f32 simultaneously, packed
    2-per-bank across all 8 banks (half-bank sharing: the bank's first
    matmul uses start=True, which marks the whole 2KB zero-region
    pending-zero; the partner chain's first matmul uses start=False and
    overwrites its still-pending half; the bank's last matmul carries
    stop=True). Chains 12..15 live in the banks that host mm1 first, so
    their products lag LAG_TR steps behind the stream.
  - Drain: relu(d * psum) split ACT/DVE, stores batched 4 row-tiles per
    dispatch alternating the sync (HWDGE) and gpsimd (SWDGE) queues.
"""

from contextlib import ExitStack

import numpy as np
import ml_dtypes

import concourse.bacc as bacc
import concourse.mybir as mybir
import concourse.tile as tile
from concourse.bass_utils import run_bass_kernel_spmd
from concourse.masks import make_identity

B = 8
N = 2048
F = 256
P = 128
NT = N // P  # 16 row tiles
FT = F // P  # 2 feature chunks
F32 = mybir.dt.float32
BF16 = mybir.dt.bfloat16
COPY = mybir.ActivationFunctionType.Copy
RELU = mybir.ActivationFunctionType.Relu
PF = 5  # A tiles (of each kind) prefetched ahead
STORE_BATCH = 4
WARMUP_MMS = 70  # identity matmuls leading the PE queue: HAM warm-up
LAG_TR = 4  # steps by which chains 12..15 lag (their banks host mm1 first)
RED_AHEAD = 2  # degree reduces run this many steps ahead of their y


def _emit(ctx: ExitStack, tc: tile.TileContext, AT, AN, XT, WTB, BIASB, OUT):
    nc = tc.nc

    const = ctx.enter_context(tc.tile_pool(name="const", bufs=1))
    at_stage = ctx.enter_context(tc.tile_pool(name="at_stage", bufs=PF + 7))
    an_stage = ctx.enter_context(tc.tile_pool(name="an_stage", bufs=PF + 2))
    scr = ctx.enter_context(tc.tile_pool(name="scr", bufs=3))
    outstage = ctx.enter_context(tc.tile_pool(name="outstage", bufs=4))
    psum_acc = ctx.enter_context(tc.tile_pool(name="psum_acc", bufs=6, space="PSUM"))
    psum_tr = ctx.enter_context(tc.tile_pool(name="psum_tr", bufs=2, space="PSUM"))

    # ---- head DMA, one queue, critical-path order (XT feeds mm1 first) ----
    xt_sb = const.tile([P, FT * N], BF16, tag="xt")
    nc.sync.dma_start(
        out=xt_sb[:, :].rearrange("p (c n) -> p c n", c=FT),
        in_=XT.rearrange("(c p) n -> p c n", p=P),
    )
    wt_sb = const.tile([P, FT * F], BF16, tag="wt")
    nc.sync.dma_start(
        out=wt_sb[:, :].rearrange("p (c f) -> p c f", c=FT),
        in_=WTB.rearrange("(c p) f -> p c f", p=P),
    )
    b_bf = const.tile([1, F], BF16, tag="bbf")
    nc.sync.dma_start(out=b_bf[:, :], in_=BIASB[:, :])

    at_tiles = {}
    an_tiles = {}

    def emit_load(k):
        an_tiles[k] = an_stage.tile([P, N], BF16, tag="an", name=f"an_{k}")
        nc.sync.dma_start(out=an_tiles[k][:, :], in_=AN[k * P : (k + 1) * P, :])
        at_tiles[k] = at_stage.tile([P, N], BF16, tag="at", name=f"at_{k}")
        nc.sync.dma_start(out=at_tiles[k][:, :], in_=AT[k * P : (k + 1) * P, :])

    for k in range(PF):
        emit_load(k)

    ones_bf = const.tile([1, P], BF16, tag="ones")
    nc.vector.memset(ones_bf[:, :], 1.0)
    ident = const.tile([P, P], BF16, tag="ident")
    make_identity(nc, ident[:, :])

    deg = const.tile([P, NT], F32, tag="deg")
    rec = const.tile([P, NT], F32, tag="rec")
    dinv = const.tile([P, NT], F32, tag="dinv")
    t_big = const.tile([P, NT * F], F32, tag="t")
    y_big = const.tile([P, NT * F], BF16, tag="y")

    # warm-up leads the PE queue: it runs while the head loads are in flight,
    # trips the HAM un-throttle, and ends roughly when mm1's inputs land
    warm = psum_acc.tile([P, 2 * F], F32, tag="acc", name="warm")
    for _ in range(WARMUP_MMS):
        nc.tensor.matmul(
            warm[:, 0:P], ident[:, :], ident[:, :], start=True, stop=True
        )

    def emit_reduce(k, an_t):
        # degree row-sums, split across DVE (even k) and ACT (odd k)
        sc = scr.tile([P, N], BF16, tag="sc", name=f"sc_{k}")
        if k % 2 == 0:
            nc.vector.tensor_scalar(
                out=sc[:, :],
                in0=an_t[:, :],
                scalar1=0.0,
                scalar2=None,
                op0=mybir.AluOpType.add,
                op1=mybir.AluOpType.add,
                accum_out=deg[:, k : k + 1],
            )
        else:
            nc.scalar.activation(
                sc[:, :], an_t[:, :], COPY, accum_out=deg[:, k : k + 1]
            )
        nc.vector.reciprocal(rec[:, k : k + 1], deg[:, k : k + 1])

    # ---- mm1 pair-chains (two t-tiles per PSUM bank) ----
    tpp = {}

    def emit_mm1_pair(pj):
        tpp[pj] = psum_tr.tile([P, 2 * F], F32, tag="tr", name=f"tpp_{pj}")
        for jj in range(2):
            j = 2 * pj + jj
            reg = tpp[pj][:, jj * F : (jj + 1) * F]
            nc.tensor.matmul(
                reg, ones_bf[:, :], b_bf[:, :], start=(jj == 0), stop=False
            )
            for phi in range(FT):
                nc.tensor.matmul(
                    reg,
                    xt_sb[:, phi * N + j * P : phi * N + (j + 1) * P],
                    wt_sb[:, phi * F : (phi + 1) * F],
                    start=False,
                    stop=(jj == 1 and phi == FT - 1),
                )

    def emit_pair_drain(pj):
        dst = t_big[:, 2 * pj * F : (2 * pj + 2) * F]
        if pj % 2 == 0:
            nc.scalar.copy(dst, tpp[pj][:, :])
        else:
            nc.vector.tensor_copy(dst, tpp[pj][:, :])

    emit_mm1_pair(0)
    emit_mm1_pair(1)
    emit_pair_drain(0)
    emit_pair_drain(1)

    # prime the reduce-ahead pipeline on DVE (both, so ACT's FIFO stays clear
    # for the first sqrt/y)
    sc0 = scr.tile([P, N], BF16, tag="sc", name="sc_p0")
    nc.vector.tensor_scalar(
        out=sc0[:, :], in0=an_tiles[0][:, :], scalar1=0.0, scalar2=None,
        op0=mybir.AluOpType.add, op1=mybir.AluOpType.add,
        accum_out=deg[:, 0:1],
    )
    nc.vector.reciprocal(rec[:, 0:1], deg[:, 0:1])
    sc1 = scr.tile([P, N], BF16, tag="sc", name="sc_p1")
    nc.vector.tensor_scalar(
        out=sc1[:, :], in0=an_tiles[1][:, :], scalar1=0.0, scalar2=None,
        op0=mybir.AluOpType.add, op1=mybir.AluOpType.add,
        accum_out=deg[:, 1:2],
    )
    nc.vector.reciprocal(rec[:, 1:2], deg[:, 1:2])

    # ---- all 16 accumulation chains, 2 per bank ----
    acc_banks = [
        psum_acc.tile([P, 2 * F], F32, tag="acc", name=f"accbank_{b_}")
        for b_ in range(6)
    ]
    cbank = {}  # allocated after the last mm1 pair rotates through

    def acc_region(mu):
        half = (mu % 2) * F
        if mu < 12:
            return acc_banks[mu // 2][:, half : half + F]
        return cbank[12 if mu < 14 else 14][:, half : half + F]

    ostiles = {}

    def emit_drain(mu):
        bi = mu // STORE_BATCH
        if bi not in ostiles:
            ostiles[bi] = outstage.tile(
                [P, STORE_BATCH * F], F32, tag="os", name=f"os_{bi}"
            )
        j = mu % STORE_BATCH
        dst = ostiles[bi][:, j * F : (j + 1) * F]
        if mu % 2 == 0:
            nc.scalar.activation(
                dst, acc_region(mu), RELU, scale=dinv[:, mu : mu + 1]
            )
        else:
            nc.vector.tensor_scalar(
                out=dst,
                in0=acc_region(mu),
                scalar1=dinv[:, mu : mu + 1],
                scalar2=0.0,
                op0=mybir.AluOpType.mult,
                op1=mybir.AluOpType.max,
            )
        if j == STORE_BATCH - 1:
            lo = bi * STORE_BATCH
            q = nc.sync if (bi % 2 == 0) else nc.gpsimd
            q.dma_start(
                out=OUT.rearrange("(m p) f -> p m f", p=P)[:, lo : lo + STORE_BATCH, :],
                in_=ostiles[bi][:, :].rearrange("p (m f) -> p m f", m=STORE_BATCH),
            )

    def emit_products(k, mus):
        for mu in mus:
            nc.tensor.matmul(
                acc_region(mu),
                at_tiles[k][:, mu * P : (mu + 1) * P],
                y_big[:, k * F : (k + 1) * F],
                start=(k == 0 and mu % 2 == 0),
                stop=(k == NT - 1 and mu % 2 == 1),
            )

    # ---- stream over the contraction index k ----
    for k in range(NT):
        if k + PF < NT:
            emit_load(k + PF)
        nc.scalar.sqrt(dinv[:, k : k + 1], rec[:, k : k + 1])
        nc.scalar.activation(
            y_big[:, k * F : (k + 1) * F],
            t_big[:, k * F : (k + 1) * F],
            COPY,
            scale=dinv[:, k : k + 1],
        )
        # remaining mm1 pairs + their drains ride the first stream steps
        if k < 3:
            emit_mm1_pair(2 * k + 2)
            emit_mm1_pair(2 * k + 3)
            emit_pair_drain(2 * k + 2)
            emit_pair_drain(2 * k + 3)
        if k == 3:
            cbank[12] = psum_tr.tile([P, 2 * F], F32, tag="tr", name="cbank12")
            cbank[14] = psum_tr.tile([P, 2 * F], F32, tag="tr", name="cbank14")
        emit_products(k, range(12))
        if k >= LAG_TR:
            emit_products(k - LAG_TR, range(12, NT))
        # reduce for step k+RED_AHEAD runs now, behind this step's sqrt/y,
        # so y latency never includes a reduce
        ka = k + RED_AHEAD
        if ka < NT:
            emit_reduce(ka, an_tiles.pop(ka))

    for k in range(NT - LAG_TR, NT):
        emit_products(k, range(12, NT))

    # ---- tail: relu(d * acc) and batched stores ----
    for mu in range(NT):
        emit_drain(mu)


_cached_nc = None


def _build():
    nc = bacc.Bacc("TRN2", target_bir_lowering=False, debug=False)
    AT = nc.dram_tensor("at", [N, N], BF16, kind="ExternalInput").ap()
    AN = nc.dram_tensor("an", [N, N], BF16, kind="ExternalInput").ap()
    XT = nc.dram_tensor("xt", [F, N], BF16, kind="ExternalInput").ap()
    WTB = nc.dram_tensor("wtb", [F, F], BF16, kind="ExternalInput").ap()
    BIASB = nc.dram_tensor("biasb", [1, F], BF16, kind="ExternalInput").ap()
    OUT = nc.dram_tensor("out", [N, F], F32, kind="ExternalOutput").ap()
    with tile.TileContext(nc) as tc:
        with ExitStack() as ctx:
            _emit(ctx, tc, AT, AN, XT, WTB, BIASB, OUT)
    nc.compile()
    return nc


def get_nc():
    global _cached_nc
    if _cached_nc is None:
        _cached_nc = _build()
    return _cached_nc


def make_in_maps(node_features, adj_matrix, W, b):
    bf16 = ml_dtypes.bfloat16
    node_features = np.asarray(node_features, dtype=np.float32)
    adj_matrix = np.asarray(adj_matrix, dtype=np.float32)
    an = adj_matrix.astype(bf16)  # [B, N, N] natural
    at = np.ascontiguousarray(an.transpose(0, 2, 1))  # [B, N, N] transposed
    xt = np.ascontiguousarray(
        node_features.astype(bf16).transpose(0, 2, 1)
    )  # [B, F, N]
    wtb = np.ascontiguousarray(np.asarray(W, dtype=np.float32).T.astype(bf16))
    biasb = np.ascontiguousarray(
        np.asarray(b, dtype=np.float32).reshape(1, F).astype(bf16)
    )
    return [
        {
            "at": np.ascontiguousarray(at[c]),
            "an": np.ascontiguousarray(an[c]),
            "xt": xt[c],
            "wtb": wtb,
            "biasb": biasb,
        }
        for c in range(B)
    ]


def kernel(node_features, adj_matrix, W, b):
    nc = get_nc()
    in_maps = make_in_maps(node_features, adj_matrix, W, b)
    res = run_bass_kernel_spmd(nc, in_maps, core_ids=list(range(B)))
    return np.stack([r["out"] for r in res.results], axis=0)


# revision 29
# speedup vs baseline: 1.0222x; 1.0222x over previous
"""GCN layer kernel for Trainium2 (Bass/Tile), data-parallel over batch.

Reference computation (per batch element):
    deg = A.sum(-1); d = deg ** -0.5
    t   = X @ W.T + b
    out = relu(diag(d) @ A @ diag(d) @ t)

Per-core mapping (8 cores, one batch element each). Host-side staging is
layout/dtype only (transposes + bf16 rounding, the same rounding the device
matmul path would apply); all model arithmetic (degree, normalization,
matmuls, bias, relu) runs on device:
  - A is staged twice in bf16: AT (transposed, the matmul stationary) and
    AN (natural, for the on-device degree row-sums). Streaming over the
    contraction index k, AT row-tile k provides the stationary chunks for
    ALL 16 output tiles, so each step runs a uniform batch of 16 products
    (k, mu) — no triangular schedule and no on-device transposes.
  - deg row-sums split DVE/ACT (accum_out) from AN tiles; d = sqrt(1/deg).
  - t = X @ W.T + b in bf16 from host-staged XT/WT, two chains per PSUM
    bank (8 wide drains split ACT/DVE); the bias is folded in as a K=1
    ones x b product initializing each group. Pairs 0-1 run in the head
    (doubling as PE warm-up for the HAM clock gate, topped up by a few
    identity matmuls); pairs 2-7 interleave into the first stream steps.
    y[k] = d[k] * t[k] rounded to bf16 by ACT.
  - All 16 output chains accumulate in PSUM f32 simultaneously, packed
    2-per-bank across all 8 banks (half-bank sharing: the bank's first
    matmul uses start=True, which marks the whole 2KB zero-region
    pending-zero; the partner chain's first matmul uses start=False and
    overwrites its still-pending half; the bank's last matmul carries
    stop=True). Chains 12..15 live in the banks that host mm1 first, so
    their products lag LAG_TR steps behind the stream.
  - Drain: relu(d * psum) split ACT/DVE, stores batched 4 row-tiles per
    dispatch alternating the sync (HWDGE) and gpsimd (SWDGE) queues.
"""

from contextlib import ExitStack

import numpy as np
import ml_dtypes

import concourse.bacc as bacc
import concourse.mybir as mybir
import concourse.tile as tile
from concourse.bass_utils import run_bass_kernel_spmd
from concourse.masks import make_identity

B = 8
N = 2048
F = 256
P = 128
NT = N // P  # 16 row tiles
FT = F // P  # 2 feature chunks
F32 = mybir.dt.float32
BF16 = mybir.dt.bfloat16
COPY = mybir.ActivationFunctionType.Copy
RELU = mybir.ActivationFunctionType.Relu
PF = 5  # A tiles (of each kind) prefetched ahead
STORE_BATCH = 4
WARMUP_MMS = 70  # identity matmuls leading the PE queue: HAM warm-up
LAG_TR = 4  # steps by which chains 12..15 lag (their banks host mm1 first)
RED_AHEAD = 2  # degree reduces run this many steps ahead of their y


def _emit(ctx: ExitStack, tc: tile.TileContext, AT, AN, XT, WTB, BIASB, OUT):
    nc = tc.nc

    const = ctx.enter_context(tc.tile_pool(name="const", bufs=1))
    at_stage = ctx.enter_context(tc.tile_pool(name="at_stage", bufs=PF + 7))
    an_stage = ctx.enter_context(tc.tile_pool(name="an_stage", bufs=PF + 2))
    scr = ctx.enter_context(tc.tile_pool(name="scr", bufs=3))
    outstage = ctx.enter_context(tc.tile_pool(name="outstage", bufs=4))
    psum_acc = ctx.enter_context(tc.tile_pool(name="psum_acc", bufs=6, space="PSUM"))
    psum_tr = ctx.enter_context(tc.tile_pool(name="psum_tr", bufs=2, space="PSUM"))

    # ---- head DMA, one queue, critical-path order (XT feeds mm1 first) ----
    xt_sb = const.tile([P, FT * N], BF16, tag="xt")
    nc.sync.dma_start(
        out=xt_sb[:, :].rearrange("p (c n) -> p c n", c=FT),
        in_=XT.rearrange("(c p) n -> p c n", p=P),
    )
    wt_sb = const.tile([P, FT * F], BF16, tag="wt")
    nc.sync.dma_start(
        out=wt_sb[:, :].rearrange("p (c f) -> p c f", c=FT),
        in_=WTB.rearrange("(c p) f -> p c f", p=P),
    )
    b_bf = const.tile([1, F], BF16, tag="bbf")
    nc.sync.dma_start(out=b_bf[:, :], in_=BIASB[:, :])

    at_tiles = {}
    an_tiles = {}

    def emit_load(k):
        an_tiles[k] = an_stage.tile([P, N], BF16, tag="an", name=f"an_{k}")
        nc.sync.dma_start(out=an_tiles[k][:, :], in_=AN[k * P : (k + 1) * P, :])
        at_tiles[k] = at_stage.tile([P, N], BF16, tag="at", name=f"at_{k}")
        nc.sync.dma_start(out=at_tiles[k][:, :], in_=AT[k * P : (k + 1) * P, :])

    for k in range(PF):
        emit_load(k)

    ones_bf = const.tile([1, P], BF16, tag="ones")
    nc.vector.memset(ones_bf[:, :], 1.0)
    ident = const.tile([P, P], BF16, tag="ident")
    make_identity(nc, ident[:, :])

    dega = const.tile([P, NT], F32, tag="dega")
    degb = const.tile([P, NT], F32, tag="degb")
    rec = const.tile([P, NT], F32, tag="rec")
    dinv = const.tile([P, NT], F32, tag="dinv")
    t_big = const.tile([P, NT * F], F32, tag="t")
    y_big = const.tile([P, NT * F], BF16, tag="y")

    # warm-up leads the PE queue: it runs while the head loads are in flight,
    # trips the HAM un-throttle, and ends roughly when mm1's inputs land
    warm = psum_acc.tile([P, 2 * F], F32, tag="acc", name="warm")
    for _ in range(WARMUP_MMS):
        nc.tensor.matmul(
            warm[:, 0:P], ident[:, :], ident[:, :], start=True, stop=True
        )

    H = N // 2

    def emit_reduce(k, an_t):
        # degree row-sums, each tile split half on DVE and half on ACT
        sc = scr.tile([P, N], BF16, tag="sc", name=f"sc_{k}")
        nc.vector.tensor_scalar(
            out=sc[:, 0:H],
            in0=an_t[:, 0:H],
            scalar1=0.0,
            scalar2=None,
            op0=mybir.AluOpType.add,
            op1=mybir.AluOpType.add,
            accum_out=dega[:, k : k + 1],
        )
        nc.scalar.activation(
            sc[:, H:N], an_t[:, H:N], COPY, accum_out=degb[:, k : k + 1]
        )
        nc.vector.tensor_add(
            rec[:, k : k + 1], dega[:, k : k + 1], degb[:, k : k + 1]
        )
        nc.vector.reciprocal(rec[:, k : k + 1], rec[:, k : k + 1])

    # ---- mm1 pair-chains (two t-tiles per PSUM bank) ----
    tpp = {}

    def emit_mm1_pair(pj):
        tpp[pj] = psum_tr.tile([P, 2 * F], F32, tag="tr", name=f"tpp_{pj}")
        for jj in range(2):
            j = 2 * pj + jj
            reg = tpp[pj][:, jj * F : (jj + 1) * F]
            nc.tensor.matmul(
                reg, ones_bf[:, :], b_bf[:, :], start=(jj == 0), stop=False
            )
            for phi in range(FT):
                nc.tensor.matmul(
                    reg,
                    xt_sb[:, phi * N + j * P : phi * N + (j + 1) * P],
                    wt_sb[:, phi * F : (phi + 1) * F],
                    start=False,
                    stop=(jj == 1 and phi == FT - 1),
                )

    def emit_pair_drain(pj):
        dst = t_big[:, 2 * pj * F : (2 * pj + 2) * F]
        if pj % 2 == 0:
            nc.scalar.copy(dst, tpp[pj][:, :])
        else:
            nc.vector.tensor_copy(dst, tpp[pj][:, :])

    # prime the reduce-ahead pipeline before any mm1 drains can block a queue
    emit_reduce(0, an_tiles[0])
    emit_reduce(1, an_tiles[1])

    emit_mm1_pair(0)
    emit_mm1_pair(1)
    emit_pair_drain(0)

    # ---- all 16 accumulation chains, 2 per bank ----
    acc_banks = [
        psum_acc.tile([P, 2 * F], F32, tag="acc", name=f"accbank_{b_}")
        for b_ in range(6)
    ]
    cbank = {}  # allocated after the last mm1 pair rotates through

    def acc_region(mu):
        half = (mu % 2) * F
        if mu < 12:
            return acc_banks[mu // 2][:, half : half + F]
        return cbank[12 if mu < 14 else 14][:, half : half + F]

    ostiles = {}

    def emit_drain(mu):
        bi = mu // STORE_BATCH
        if bi not in ostiles:
            ostiles[bi] = outstage.tile(
                [P, STORE_BATCH * F], F32, tag="os", name=f"os_{bi}"
            )
        j = mu % STORE_BATCH
        dst = ostiles[bi][:, j * F : (j + 1) * F]
        if mu % 2 == 0:
            nc.scalar.activation(
                dst, acc_region(mu), RELU, scale=dinv[:, mu : mu + 1]
            )
        else:
            nc.vector.tensor_scalar(
                out=dst,
                in0=acc_region(mu),
                scalar1=dinv[:, mu : mu + 1],
                scalar2=0.0,
                op0=mybir.AluOpType.mult,
                op1=mybir.AluOpType.max,
            )
        if j == STORE_BATCH - 1:
            lo = bi * STORE_BATCH
            q = nc.sync if (bi % 2 == 0) else nc.gpsimd
            q.dma_start(
                out=OUT.rearrange("(m p) f -> p m f", p=P)[:, lo : lo + STORE_BATCH, :],
                in_=ostiles[bi][:, :].rearrange("p (m f) -> p m f", m=STORE_BATCH),
            )

    def emit_products(k, mus):
        for mu in mus:
            nc.tensor.matmul(
                acc_region(mu),
                at_tiles[k][:, mu * P : (mu + 1) * P],
                y_big[:, k * F : (k + 1) * F],
                start=(k == 0 and mu % 2 == 0),
                stop=(k == NT - 1 and mu % 2 == 1),
            )

    # ---- stream over the contraction index k ----
    for k in range(NT):
        if k + PF < NT:
            emit_load(k + PF)
        nc.scalar.sqrt(dinv[:, k : k + 1], rec[:, k : k + 1])
        nc.scalar.activation(
            y_big[:, k * F : (k + 1) * F],
            t_big[:, k * F : (k + 1) * F],
            COPY,
            scale=dinv[:, k : k + 1],
        )
        # remaining mm1 pairs ride the first stream steps (PE fill-in)
        if k < 3:
            emit_mm1_pair(2 * k + 2)
            emit_mm1_pair(2 * k + 3)
        if k == 3:
            cbank[12] = psum_tr.tile([P, 2 * F], F32, tag="tr", name="cbank12")
            cbank[14] = psum_tr.tile([P, 2 * F], F32, tag="tr", name="cbank14")
        emit_products(k, range(12))
        if k >= LAG_TR:
            emit_products(k - LAG_TR, range(12, NT))
        # reduce for step k+RED_AHEAD runs now, behind this step's sqrt/y,
        # so y latency never includes a reduce
        ka = k + RED_AHEAD
        if ka < NT:
            emit_reduce(ka, an_tiles.pop(ka))
        # mm1 pair drains trail their chains closely so the tr-bank rotation
        # (and the cbank allocs behind it) never blocks the PE for long
        for pj in {1: (1, 2), 2: (3, 4), 3: (5, 6), 4: (7,)}.get(k, ()):
            emit_pair_drain(pj)

    for k in range(NT - LAG_TR, NT):
        emit_products(k, range(12, NT))

    # ---- tail: relu(d * acc) and batched stores ----
    for mu in range(NT):
        emit_drain(mu)


_cached_nc = None


def _build():
    nc = bacc.Bacc("TRN2", target_bir_lowering=False, debug=False)
    AT = nc.dram_tensor("at", [N, N], BF16, kind="ExternalInput").ap()
    AN = nc.dram_tensor("an", [N, N], BF16, kind="ExternalInput").ap()
    XT = nc.dram_tensor("xt", [F, N], BF16, kind="ExternalInput").ap()
    WTB = nc.dram_tensor("wtb", [F, F], BF16, kind="ExternalInput").ap()
    BIASB = nc.dram_tensor("biasb", [1, F], BF16, kind="ExternalInput").ap()
    OUT = nc.dram_tensor("out", [N, F], F32, kind="ExternalOutput").ap()
    with tile.TileContext(nc) as tc:
        with ExitStack() as ctx:
            _emit(ctx, tc, AT, AN, XT, WTB, BIASB, OUT)
    nc.compile()
    return nc


def get_nc():
    global _cached_nc
    if _cached_nc is None:
        _cached_nc = _build()
    return _cached_nc


def make_in_maps(node_features, adj_matrix, W, b):
    bf16 = ml_dtypes.bfloat16
    node_features = np.asarray(node_features, dtype=np.float32)
    adj_matrix = np.asarray(adj_matrix, dtype=np.float32)
    an = adj_matrix.astype(bf16)  # [B, N, N] natural
    at = np.ascontiguousarray(an.transpose(0, 2, 1))  # [B, N, N] transposed
    xt = np.ascontiguousarray(
        node_features.astype(bf16).transpose(0, 2, 1)
    )  # [B, F, N]
    wtb = np.ascontiguousarray(np.asarray(W, dtype=np.float32).T.astype(bf16))
    biasb = np.ascontiguousarray(
        np.asarray(b, dtype=np.float32).reshape(1, F).astype(bf16)
    )
    return [
        {
            "at": np.ascontiguousarray(at[c]),
            "an": np.ascontiguousarray(an[c]),
            "xt": xt[c],
            "wtb": wtb,
            "biasb": biasb,
        }
        for c in range(B)
    ]


def kernel(node_features, adj_matrix, W, b):
    nc = get_nc()
    in_maps = make_in_maps(node_features, adj_matrix, W, b)
    res = run_bass_kernel_spmd(nc, in_maps, core_ids=list(range(B)))
    return np.stack([r["out"] for r in res.results], axis=0)
